# revision 1
# baseline (speedup 1.0000x reference)
"""Trainium2 Bass kernel: grouped similarity-gating normalization.

Reference computation (per batch b, group g, cpg=64 channels, hw=784):
    means[c]  = mean_hw(x[c, :])
    s[hw]     = sum_c x[c, hw] * means[c]
    t         = (s - mean(s)) * rsqrt(var(s) + eps)
    gate      = sigmoid(t * weight[g] + bias[g])
    out[c,hw] = x[c, hw] * gate[hw]

Sharding: data-parallel over batch B=64 across 8 cores (8 batches/core).

Per-core layout: one SBUF tile [128, 4, 784] per batch holds channels
c = 4*p + j (p = partition, j = free chunk) -> contiguous 1.6MB DMAs, and
group(c) = c//64 = p//16, i.e. each group owns a 16-partition band.

  - channel means via one DVE reduce (innermost axis of [128,4,784])
  - s (contraction over channels) via PE: 4 accumulating matmuls with
    lhsT[p, g] = means[p,j] masked to group bands (constant 0/1 indicator
    m8 times means). An extra N=1 matmul column with rhs=means gives
    mu = mean(s) = sum_c means[c]^2 for free.
  - stats on ScalarE: Square+accum_out -> sum(s^2); gate computed as
    sigmoid(s*a + c) in one activation with per-partition scale/bias APs,
    where a = rstd*weight[g], c = bias[g] - mu*a.
  - gate broadcast to the 128 partitions via PE with the transposed
    indicator (mt), then one DVE multiply (j-broadcast AP), DMA out.
"""

import sys

if "/opt/trn_rl_repo" not in sys.path:
    sys.path.insert(0, "/opt/trn_rl_repo")

from contextlib import ExitStack

import numpy as np

import concourse.bacc as bacc
import concourse.bass as bass
import concourse.tile as tile
from concourse import mybir
from concourse.bass_utils import run_bass_kernel_spmd

B, C, H, W = 64, 512, 28, 28
G = 8
HW = H * W          # 784
NCORES = 8
BLOC = B // NCORES  # 8 batches per core
NP = 128            # SBUF partitions
NJ = C // NP        # 4 channel chunks per partition (c = NJ*p + j)
PBAND = NP // G     # 16 partitions per group
EPS = 1e-5
F32 = mybir.dt.float32
MMCHUNK = 512       # max fp32 moving free dim per matmul

_cache: dict = {}

# implementation choices (bisectable)
OUT_ENGINE = "sync"  # "scalar" or "sync" HWDGE ring for output DMAs
MUL_J3 = "gpsimd"    # "gpsimd" or "vector" for the last gating multiply
REDUCE_MODE = "split"  # "split" (DVE j01 + ACT copy-accum j23) or "dve" (one reduce)
# NOTE: tensor_tensor_reduce (custom DVE ucode op) wedges the device under
# the axon/PJRT runtime (NRT_EXEC_UNIT_UNRECOVERABLE) -- keep "plain".
SQ2_MODE = "plain"   # "ttr" (tensor_tensor_reduce) or "plain" (mul + reduce)
MM_DTYPE = "fp32"    # "fp32" (2-pass, exact) or "fp32r" (1-pass, rounded ~tf32)
N_POOL_MULS = 2      # how many of the 4 gating multiplies run on GpSimd
S_MODE = "pe4r"      # "pe4": 4 PE contraction passes (all fp32)
                     # "hybrid": PE j0/j1 + z23 on DVE, band-summed on PE (fp32)
                     # "zr": z = sum_j means_j*x_j as two DVE half-chains,
                     #       rounded to fp32r, band-summed on PE with an exact
                     #       0/1 fp32r indicator at 1 cyc/row (4x faster PE)
                     # "pe4r": pe4 structure, but PE-feeding tiles declared
                     #       float32r (raw fp32 bits; PE truncates ~tf32).
                     #       No extra DVE passes; x/output path stays exact.


def _emit(tc, nc, xs, m8, wv, bv, ys):
    AF = mybir.ActivationFunctionType
    OP = mybir.AluOpType
    PREF = 3  # input prefetch depth (batches)
    with ExitStack() as ctx:
        consts = ctx.enter_context(tc.tile_pool(name="consts", bufs=1))
        xpool = ctx.enter_context(tc.tile_pool(name="xpool", bufs=BLOC))
        mpool = ctx.enter_context(tc.tile_pool(name="mpool", bufs=3))
        vpool = ctx.enter_context(tc.tile_pool(name="vpool", bufs=4))
        gpool = ctx.enter_context(tc.tile_pool(name="gpool", bufs=3))
        spsum = ctx.enter_context(tc.tile_pool(name="spsum", bufs=2, space="PSUM"))
        opool = ctx.enter_context(tc.tile_pool(name="opool", bufs=2))

        # m8 input now carries the [NP, NP] block-banded 0/1 indicator
        # M16[p, q] = (p//PBAND == q//PBAND); wv/bv are 16x-replicated [NP, 1]
        m16_sb = consts.tile([NP, NP], F32)
        nc.sync.dma_start(out=m16_sb[:], in_=m8[:])
        # fp32r copy for the zr band-sum (0/1 is exactly representable)
        m16r_sb = consts.tile([NP, NP], mybir.dt.float32r)
        nc.sync.dma_start(out=m16r_sb[:], in_=m8[:].bitcast(mybir.dt.float32r))
        wv_sb = consts.tile([NP, 1], F32)
        nc.sync.dma_start(out=wv_sb[:], in_=wv[:])
        bv_sb = consts.tile([NP, 1], F32)
        nc.sync.dma_start(out=bv_sb[:], in_=bv[:])
        eps_sb = consts.tile([NP, 1], F32)
        nc.vector.memset(eps_sb[:], EPS)

        xts = {}
        state = {}

        XT_DT = mybir.dt.float32r if S_MODE == "pe4r" else F32

        def dma_in(b):
            # HW+2 free elems per j: columns HW:HW+2 later hold means so the
            # matmul's second chunk also accumulates mu = sum(means^2) for free
            # (2 columns to keep fp32r chunk widths even)
            xt = xpool.tile([NP, NJ, HW + 2], XT_DT)
            # two chunks so the means reduce can start on the first half
            nc.sync.dma_start(out=xt[:, 0:2, 0:HW], in_=xs[b, :, 0:2, :])
            nc.sync.dma_start(out=xt[:, 2:4, 0:HW], in_=xs[b, :, 2:4, :])
            xts[b] = xt

        def phase1_pe4r(b):
            # raw sums (j0 on DVE, j1-3 on ACT), fused means-column stash,
            # lhsT = m16 * sums_j/HW written straight into fp32r tiles
            xt = xts[b]
            xf = lambda ap: ap.bitcast(F32)
            sums = mpool.tile([NP, NJ], F32, tag="sums")
            nc.vector.reduce_sum(
                out=sums[:, 0:1], in_=xf(xt[:, 0:1, 0:HW]), axis=mybir.AxisListType.X
            )
            cps = gpool.tile([NP, HW], F32, tag="cps")
            for j in (1, 2, 3):
                nc.scalar.activation(
                    out=cps[:], in_=xf(xt[:, j, 0:HW]), func=AF.Copy,
                    accum_out=sums[:, j : j + 1],
                )
            # columns HW:HW+2 of xt get sums_j; lhsT carries means_j, so the
            # matmul's mu column accumulates sum_j means_j*sums_j = HW*mu
            # (f32r-typed output so the fp32r-matmult producer check passes)
            nc.vector.tensor_copy(
                xt[:, :, HW : HW + 2],
                sums[:].unsqueeze(2).to_broadcast([NP, NJ, 2]),
            )
            lhsT = mpool.tile([NP, NJ, NP], mybir.dt.float32r, tag="lhsT")
            for j in range(NJ):
                nc.vector.tensor_scalar(
                    out=lhsT[:, j, :], in0=m16r_sb[:],
                    scalar1=sums[:, j : j + 1], scalar2=1.0 / HW,
                    op0=OP.mult, op1=OP.mult,
                )
            state[b] = (lhsT, ())

        def phase1(b):
            if S_MODE == "pe4r":
                return phase1_pe4r(b)
            # means + masked lhsT (all pre-matmul per-batch prep)
            xt = xts[b]
            means = mpool.tile([NP, NJ], F32, tag="means")
            if REDUCE_MODE == "split":
                sums01 = mpool.tile([NP, 2], F32, tag="sums01")
                nc.vector.reduce_sum(out=sums01[:], in_=xt[:, 0:2, 0:HW], axis=mybir.AxisListType.X)
                nc.vector.tensor_scalar_mul(means[:, 0:2], sums01[:], 1.0 / HW)
                cps = gpool.tile([NP, HW], F32, tag="cps")
                for j in (2, 3):
                    nc.scalar.activation(
                        out=cps[:], in_=xt[:, j, 0:HW], func=AF.Copy,
                        scale=1.0 / HW, accum_out=means[:, j : j + 1],
                    )
            else:
                sums = mpool.tile([NP, NJ], F32, tag="sums")
                nc.vector.reduce_sum(out=sums[:], in_=xt[:, :, 0:HW], axis=mybir.AxisListType.X)
                nc.vector.tensor_scalar_mul(means[:], sums[:], 1.0 / HW)

            # stash means[:, j] in column HW of xt so the second matmul chunk
            # accumulates mu[g] = sum_{c in g} means_c^2 into ps[:, HW]
            for j in range(NJ):
                nc.vector.tensor_copy(
                    xt[:, j, HW : HW + 2], means[:, j : j + 1].to_broadcast([NP, 2])
                )

            if S_MODE == "zr":
                # z = sum_j means_j * x_j as two fp32 half-chains; only the
                # final writes round to fp32r (error ~2^-13 of z, not of x)
                za0 = mpool.tile([NP, HW + 2], F32, tag="za0")
                nc.vector.tensor_scalar_mul(za0[:], xt[:, 0, :], means[:, 0:1])
                zar = mpool.tile([NP, HW + 2], mybir.dt.float32r, tag="zar")
                nc.vector.scalar_tensor_tensor(
                    out=zar[:], in0=xt[:, 1, :], scalar=means[:, 1:2], in1=za0[:],
                    op0=OP.mult, op1=OP.add,
                )
                zb0 = mpool.tile([NP, HW + 2], F32, tag="zb0")
                nc.vector.tensor_scalar_mul(zb0[:], xt[:, 2, :], means[:, 2:3])
                zbr = mpool.tile([NP, HW + 2], mybir.dt.float32r, tag="zbr")
                nc.vector.scalar_tensor_tensor(
                    out=zbr[:], in0=xt[:, 3, :], scalar=means[:, 3:4], in1=zb0[:],
                    op0=OP.mult, op1=OP.add,
                )
                state[b] = (None, (zar, zbr))
                return

            # lhsT[:, j, q] = means[p, j] masked to the 16-wide band of q, so the
            # matmul emits s replicated onto all 128 PSUM partitions (M=128 is
            # free: PE cost is N-bound)
            npej = 2 if S_MODE == "hybrid" else NJ
            lhsT = mpool.tile([NP, npej, NP], F32, tag="lhsT")
            for j in range(npej):
                nc.vector.tensor_scalar_mul(lhsT[:, j, :], m16_sb[:], means[:, j : j + 1])

            if S_MODE == "hybrid":
                # z = m2*x2 + m3*x3 (incl. the means column) off the PE
                # (keep off GpSimd: its TensorScalar ucode measures ~11us/op)
                zb = mpool.tile([NP, HW + 2], F32, tag="zb")
                nc.vector.tensor_scalar_mul(zb[:], xt[:, 2, :], means[:, 2:3])
                nc.vector.scalar_tensor_tensor(
                    out=zb[:], in0=xt[:, 3, :], scalar=means[:, 3:4], in1=zb[:],
                    op0=OP.mult, op1=OP.add,
                )
                state[b] = (lhsT, (zb,))
            else:
                state[b] = (lhsT, ())

        def phase2(b):
            # s (replicated per 16-band) in cols 0:HW; replicated mu in col HW
            xt = xts[b]
            lhsT, zs = state[b]
            ps = spsum.tile([NP, HW + 2], F32)
            for c0 in range(0, HW + 2, MMCHUNK):
                c1 = min(c0 + MMCHUNK, HW + 2)
                passes = []
                if lhsT is not None:
                    npej = 2 if S_MODE == "hybrid" else NJ
                    for j in range(npej):
                        passes.append((lhsT[:, j, :], xt[:, j, c0:c1]))
                zlhs = m16r_sb if S_MODE == "zr" else m16_sb
                for z in zs:
                    passes.append((zlhs[:], z[:, c0:c1]))
                for k, (lw, rw) in enumerate(passes):
                    st = dict(start=(k == 0), stop=(k == len(passes) - 1))
                    nc.tensor.matmul(ps[:, c0:c1], lw, rw, **st)
            state[b] = ps

        def phase3(b):
            # stats + gate (everything already replicated on 128 partitions)
            ps = state[b]
            nmu = vpool.tile([NP, 1], F32, tag="nmu")
            mu_scale = -1.0 / HW if S_MODE == "pe4r" else -1.0
            nc.vector.tensor_scalar_mul(nmu[:], ps[:, HW : HW + 1], mu_scale)
            sq = gpool.tile([NP, HW], F32, tag="sq")
            hwvar = vpool.tile([NP, 1], F32, tag="hwvar")
            nc.scalar.activation(
                out=sq[:], in_=ps[:, 0:HW], func=AF.Square, bias=nmu[:], accum_out=hwvar[:]
            )
            std = vpool.tile([NP, 1], F32, tag="std")
            nc.scalar.activation(
                out=std[:], in_=hwvar[:], func=AF.Sqrt, scale=1.0 / HW, bias=eps_sb[:]
            )
            rstd = vpool.tile([NP, 1], F32, tag="rstd")
            nc.vector.reciprocal(rstd[:], std[:])
            a_t = vpool.tile([NP, 1], F32, tag="a_t")
            nc.vector.tensor_mul(a_t[:], rstd[:], wv_sb[:])
            c_t = vpool.tile([NP, 1], F32, tag="c_t")
            nc.vector.scalar_tensor_tensor(
                out=c_t[:], in0=nmu[:], scalar=a_t[:], in1=bv_sb[:],
                op0=OP.mult, op1=OP.add,
            )
            gate = gpool.tile([NP, HW], F32, tag="gate")
            nc.scalar.activation(
                out=gate[:], in_=ps[:, 0:HW], func=AF.Sigmoid, bias=c_t[:], scale=a_t[:]
            )
            state[b] = gate[:]

        def phase4(b):
            # gating multiply + store.  With fp32r xt tiles the product goes to
            # a separate fp32 tile (walrus's fp32r producer check is
            # order-blind, so in-place writes through an f32 view are rejected)
            xt = xts.pop(b)
            bg_ap = state.pop(b)
            npool = N_POOL_MULS if MUL_J3 == "gpsimd" else 0
            if XT_DT != F32:
                ot = opool.tile([NP, NJ, HW], F32)
                for j in range(NJ):
                    eng = nc.gpsimd if j >= NJ - npool else nc.vector
                    eng.tensor_mul(ot[:, j, :], xt[:, j, 0:HW].bitcast(F32), bg_ap)
                xout = ot[:]
            else:
                for j in range(NJ):
                    eng = nc.gpsimd if j >= NJ - npool else nc.vector
                    eng.tensor_mul(xt[:, j, 0:HW], xt[:, j, 0:HW], bg_ap)
                xout = xt[:, :, 0:HW]
            if OUT_ENGINE == "scalar":
                nc.scalar.dma_start(out=ys[b], in_=xout)
            else:
                nc.sync.dma_start(out=ys[b], in_=xout)
            if b + PREF < BLOC:
                dma_in(b + PREF)

        # software-pipelined emission: each engine's stream sees work in
        # data-readiness order, so in-order engines never head-of-line block
        for b in range(PREF):
            dma_in(b)
        phase1(0)
        phase2(0)
        for b in range(BLOC):
            if b + 1 < BLOC:
                phase1(b + 1)
            phase3(b)
            if b + 1 < BLOC:
                phase2(b + 1)
            phase4(b)


def _build_nc():
    nc = bacc.Bacc("TRN2", debug=False)
    xs_dt = mybir.dt.float32r if S_MODE == "pe4r" else F32
    xs = nc.dram_tensor("xs", [BLOC, NP, NJ, HW], xs_dt, kind="ExternalInput")
    m8 = nc.dram_tensor("m8", [NP, NP], F32, kind="ExternalInput")
    wv = nc.dram_tensor("wv", [NP, 1], F32, kind="ExternalInput")
    bv = nc.dram_tensor("bv", [NP, 1], F32, kind="ExternalInput")
    ys = nc.dram_tensor("ys", [BLOC, NP, NJ, HW], F32, kind="ExternalOutput")
    with tile.TileContext(nc) as tc:
        _emit(tc, nc, xs, m8, wv, bv, ys)
    nc.compile()
    return nc


def get_nc():
    if "nc" not in _cache:
        _cache["nc"] = _build_nc()
    return _cache["nc"]


def make_in_maps(x, weight, bias):
    x = np.ascontiguousarray(np.asarray(x, dtype=np.float32))
    weight = np.asarray(weight, dtype=np.float32).reshape(G)
    bias = np.asarray(bias, dtype=np.float32).reshape(G)
    # [core, b, p, j, hw] with c = NJ*p + j
    xs = x.reshape(NCORES, BLOC, NP, NJ, HW)
    band = np.arange(NP) // PBAND
    m8 = (band[:, None] == band[None, :]).astype(np.float32)  # [NP, NP] indicator
    wv = np.ascontiguousarray(np.repeat(weight, PBAND)[:, None])
    bv = np.ascontiguousarray(np.repeat(bias, PBAND)[:, None])
    return [
        {"xs": np.ascontiguousarray(xs[i]), "m8": m8, "wv": wv, "bv": bv}
        for i in range(NCORES)
    ]


def run(x, weight, bias, trace=False, **spmd_kwargs):
    nc = get_nc()
    in_maps = make_in_maps(x, weight, bias)
    res = run_bass_kernel_spmd(
        nc, in_maps, core_ids=list(range(NCORES)), trace=trace, **spmd_kwargs
    )
    out = np.stack([res.results[i]["ys"] for i in range(NCORES)])
    return out.reshape(B, C, H, W), res


def kernel(x, weight, bias, groups=G, **_ignored):
    assert int(groups) == G
    out, _ = run(x, weight, bias, trace=False)
    return out



# revision 2
# speedup vs baseline: 1.0760x; 1.0760x over previous
"""Trainium2 Bass kernel: grouped similarity-gating normalization.

Reference computation (per batch b, group g, cpg=64 channels, hw=784):
    means[c]  = mean_hw(x[c, :])
    s[hw]     = sum_c x[c, hw] * means[c]
    t         = (s - mean(s)) * rsqrt(var(s) + eps)
    gate      = sigmoid(t * weight[g] + bias[g])
    out[c,hw] = x[c, hw] * gate[hw]

Sharding: data-parallel over batch B=64 across 8 cores (8 batches/core).

Per-core layout: one SBUF tile [128, 4, 784] per batch holds channels
c = 4*p + j (p = partition, j = free chunk) -> contiguous 1.6MB DMAs, and
group(c) = c//64 = p//16, i.e. each group owns a 16-partition band.

v2 design (memory-regime; HBM floor ~54us/core with fp16 output):
  - input DMAs ride the scalar (ACT) HWDGE ring, output DMAs the sync
    (SP) ring -> no head-of-line blocking between loads and stores.
  - channel sums: j0 via one DVE reduce, j1-3 via ACT Copy+accum_out
    (copy lives in the sigmoid table set -> no ACT table swaps at all).
  - s via PE: 4 accumulating fp32r matmuls with lhsT = indicator*means.
  - mean/var of s via DVE bn_stats/bn_aggr straight from PSUM (exact:
    4 equal-count sub-groups of 196).
  - rstd on DVE: int32 magic-constant seed (shift/xor/add) + 3 Newton
    iterations -> no Sqrt on ACT, so zero ACT_TABLE_LOADs in steady
    state (all ACT funcs sit in sigmoid_and_others).
  - gate = sigmoid(s*a + c) one ACT op with per-partition scale/bias.
  - gating multiply split DVE (j0,j1) / GpSimd (j2,j3), each as ONE
    broadcast-middle-dim tensor_tensor writing fp16 -> output HBM
    traffic halved; host upcasts to fp32 (rel err ~4e-4 << 2e-2 gate).
"""

import sys

if "/opt/trn_rl_repo" not in sys.path:
    sys.path.insert(0, "/opt/trn_rl_repo")

from contextlib import ExitStack

import numpy as np

import concourse.bacc as bacc
import concourse.tile as tile
from concourse import mybir
from concourse.bass_utils import run_bass_kernel_spmd

B, C, H, W = 64, 512, 28, 28
G = 8
HW = H * W          # 784
NCORES = 8
BLOC = B // NCORES  # 8 batches per core
NP = 128            # SBUF partitions
NJ = C // NP        # 4 channel chunks per partition (c = NJ*p + j)
PBAND = NP // G     # 16 partitions per group
EPS = 1e-5
F32 = mybir.dt.float32
F32R = mybir.dt.float32r
F16 = mybir.dt.float16
I32 = mybir.dt.int32
MMCHUNK = 512       # max fp32 moving free dim per matmul (PSUM bank)
PREF = 4            # input prefetch depth (batches)
NR_ITERS = 3        # Newton iterations for rsqrt
MAGIC = 0x5F3759DF  # rsqrt seed constant

_cache: dict = {}


def _emit(tc, nc, xs, m8, wv, bv, ys):
    AF = mybir.ActivationFunctionType
    OP = mybir.AluOpType
    with ExitStack() as ctx:
        consts = ctx.enter_context(tc.tile_pool(name="consts", bufs=1))
        xpool = ctx.enter_context(tc.tile_pool(name="xpool", bufs=BLOC))
        spool = ctx.enter_context(tc.tile_pool(name="spool", bufs=4))
        lpool = ctx.enter_context(tc.tile_pool(name="lpool", bufs=3))
        cpool = ctx.enter_context(tc.tile_pool(name="cpool", bufs=2))
        gpool = ctx.enter_context(tc.tile_pool(name="gpool", bufs=3))
        vpool = ctx.enter_context(tc.tile_pool(name="vpool", bufs=20))
        spsum = ctx.enter_context(tc.tile_pool(name="spsum", bufs=2, space="PSUM"))
        opool = ctx.enter_context(tc.tile_pool(name="opool", bufs=3))

        # M16[p, q] = (p//PBAND == q//PBAND) 0/1 indicator (exact in fp32r);
        # wv/bv are the 16x-replicated per-partition weight/bias columns.
        m16r_sb = consts.tile([NP, NP], F32R)
        nc.sync.dma_start(out=m16r_sb[:], in_=m8[:].bitcast(F32R))
        wv_sb = consts.tile([NP, 1], F32)
        nc.sync.dma_start(out=wv_sb[:], in_=wv[:])
        bv_sb = consts.tile([NP, 1], F32)
        nc.sync.dma_start(out=bv_sb[:], in_=bv[:])

        xts = {}
        state = {}

        def dma_in(b):
            # two halves so phase1 can start on j0/j1 at half-arrival;
            # scalar (ACT) ring: never blocks on xpool with bufs=BLOC
            xt = xpool.tile([NP, NJ, HW], F32R)
            nc.scalar.dma_start(out=xt[:, 0:2, :], in_=xs[b, :, 0:2, :])
            nc.scalar.dma_start(out=xt[:, 2:4, :], in_=xs[b, :, 2:4, :])
            xts[b] = xt

        def phase1(b):
            # channel sums + masked lhsT (pre-matmul per-batch prep)
            xt = xts[b]
            xf = lambda ap: ap.bitcast(F32)
            sums = spool.tile([NP, NJ], F32, tag="sums")
            nc.vector.reduce_sum(
                out=sums[:, 0:1], in_=xf(xt[:, 0:1, :]), axis=mybir.AxisListType.X
            )
            cps = cpool.tile([NP, HW], F32, tag="cps")
            for j in (1, 2, 3):
                nc.scalar.activation(
                    out=cps[:], in_=xf(xt[:, j, :]), func=AF.Copy,
                    accum_out=sums[:, j : j + 1],
                )
            # lhsT[p, j, q] = indicator[p, q] * sums[p, j] / HW = masked means
            lhsT = lpool.tile([NP, NJ, NP], F32R, tag="lhsT")
            for j in range(NJ):
                nc.vector.tensor_scalar(
                    out=lhsT[:, j, :], in0=m16r_sb[:],
                    scalar1=sums[:, j : j + 1], scalar2=1.0 / HW,
                    op0=OP.mult, op1=OP.mult,
                )
            state[b] = lhsT

        def phase2(b):
            # s replicated onto each group's 16-partition band (M=128 free)
            xt = xts[b]
            lhsT = state[b]
            ps = spsum.tile([NP, HW], F32)
            for c0 in range(0, HW, MMCHUNK):
                c1 = min(c0 + MMCHUNK, HW)
                for j in range(NJ):
                    nc.tensor.matmul(
                        ps[:, c0:c1], lhsT[:, j, :], xt[:, j, c0:c1],
                        start=(j == 0), stop=(j == NJ - 1),
                    )
            state[b] = ps

        def phase3(b):
            # stats from PSUM + rstd on DVE + gate on ACT
            ps = state[b]
            st6 = vpool.tile([NP, 2, 6], F32, tag="st6")
            nc.vector.bn_stats(st6[:, 0, :], ps[:, 0 : HW // 2])
            nc.vector.bn_stats(st6[:, 1, :], ps[:, HW // 2 : HW])
            mv = vpool.tile([NP, 2], F32, tag="mv")
            nc.vector.bn_aggr(mv[:], st6[:])
            u = vpool.tile([NP, 1], F32, tag="u")
            nc.vector.tensor_scalar_add(u[:], mv[:, 1:2], EPS)
            # y0 = bits(MAGIC - (bits(u) >> 1)); K - t = (t ^ -1) + (K + 1)
            y = vpool.tile([NP, 1], F32, tag="y")
            yi = y[:].bitcast(I32)
            nc.vector.tensor_scalar(
                out=yi, in0=u[:].bitcast(I32), scalar1=1, scalar2=None,
                op0=OP.logical_shift_right,
            )
            nc.vector.tensor_scalar(
                out=yi, in0=yi, scalar1=-1, scalar2=None, op0=OP.bitwise_xor
            )
            nc.vector.tensor_scalar(
                out=yi, in0=yi, scalar1=MAGIC + 1, scalar2=None, op0=OP.add
            )
            h = vpool.tile([NP, 1], F32, tag="h")
            nc.vector.tensor_scalar_mul(h[:], u[:], -0.5)
            t = vpool.tile([NP, 1], F32, tag="t")
            for _ in range(NR_ITERS):
                nc.vector.tensor_mul(t[:], y[:], y[:])
                nc.vector.tensor_mul(t[:], t[:], h[:])
                nc.vector.scalar_tensor_tensor(
                    out=y[:], in0=t[:], scalar=1.5, in1=y[:],
                    op0=OP.add, op1=OP.mult,
                )
            a_t = vpool.tile([NP, 1], F32, tag="a_t")
            nc.vector.tensor_mul(a_t[:], y[:], wv_sb[:])
            nmu = vpool.tile([NP, 1], F32, tag="nmu")
            nc.vector.tensor_scalar_mul(nmu[:], mv[:, 0:1], -1.0)
            c_t = vpool.tile([NP, 1], F32, tag="c_t")
            nc.vector.scalar_tensor_tensor(
                out=c_t[:], in0=nmu[:], scalar=a_t[:], in1=bv_sb[:],
                op0=OP.mult, op1=OP.add,
            )
            gate = gpool.tile([NP, HW], F32, tag="gate")
            nc.scalar.activation(
                out=gate[:], in_=ps[:, 0:HW], func=AF.Sigmoid, bias=c_t[:],
                scale=a_t[:],
            )
            state[b] = gate

        def phase4(b):
            # gating multiply (fp16 out) + store halves on the sync ring
            xt = xts.pop(b)
            gate = state.pop(b)
            xf = lambda ap: ap.bitcast(F32)
            ot = opool.tile([NP, NJ, HW], F16)
            gb = lambda n: gate[:].unsqueeze(1).to_broadcast([NP, n, HW])
            nc.vector.tensor_mul(ot[:, 0:2, :], xf(xt[:, 0:2, :]), gb(2))
            nc.gpsimd.tensor_mul(ot[:, 2:4, :], xf(xt[:, 2:4, :]), gb(2))
            nc.sync.dma_start(out=ys[b, :, 0:2, :], in_=ot[:, 0:2, :])
            nc.sync.dma_start(out=ys[b, :, 2:4, :], in_=ot[:, 2:4, :])
            if b + PREF < BLOC:
                dma_in(b + PREF)

        # software-pipelined emission: each engine's stream sees work in
        # data-readiness order, so in-order engines never head-of-line block
        for b in range(min(PREF, BLOC)):
            dma_in(b)
        phase1(0)
        phase2(0)
        for b in range(BLOC):
            if b + 1 < BLOC:
                phase1(b + 1)
            phase3(b)
            if b + 1 < BLOC:
                phase2(b + 1)
            phase4(b)


def _build_nc():
    nc = bacc.Bacc("TRN2", debug=False)
    xs = nc.dram_tensor("xs", [BLOC, NP, NJ, HW], F32R, kind="ExternalInput")
    m8 = nc.dram_tensor("m8", [NP, NP], F32, kind="ExternalInput")
    wv = nc.dram_tensor("wv", [NP, 1], F32, kind="ExternalInput")
    bv = nc.dram_tensor("bv", [NP, 1], F32, kind="ExternalInput")
    ys = nc.dram_tensor("ys", [BLOC, NP, NJ, HW], F16, kind="ExternalOutput")
    with tile.TileContext(nc) as tc:
        _emit(tc, nc, xs, m8, wv, bv, ys)
    nc.compile()
    return nc


def get_nc():
    if "nc" not in _cache:
        _cache["nc"] = _build_nc()
    return _cache["nc"]


def make_in_maps(x, weight, bias):
    x = np.ascontiguousarray(np.asarray(x, dtype=np.float32))
    weight = np.asarray(weight, dtype=np.float32).reshape(G)
    bias = np.asarray(bias, dtype=np.float32).reshape(G)
    # [core, b, p, j, hw] with c = NJ*p + j
    xs = x.reshape(NCORES, BLOC, NP, NJ, HW)
    band = np.arange(NP) // PBAND
    m8 = (band[:, None] == band[None, :]).astype(np.float32)  # [NP, NP] indicator
    wv = np.ascontiguousarray(np.repeat(weight, PBAND)[:, None])
    bv = np.ascontiguousarray(np.repeat(bias, PBAND)[:, None])
    return [
        {"xs": np.ascontiguousarray(xs[i]), "m8": m8, "wv": wv, "bv": bv}
        for i in range(NCORES)
    ]


def run(x, weight, bias, trace=False, **spmd_kwargs):
    nc = get_nc()
    in_maps = make_in_maps(x, weight, bias)
    res = run_bass_kernel_spmd(
        nc, in_maps, core_ids=list(range(NCORES)), trace=trace, **spmd_kwargs
    )
    out = np.stack(
        [res.results[i]["ys"].astype(np.float32) for i in range(NCORES)]
    )
    return out.reshape(B, C, H, W), res


def kernel(x, weight, bias, groups=G, **_ignored):
    assert int(groups) == G
    out, _ = run(x, weight, bias, trace=False)
    return out


# revision 7
# speedup vs baseline: 1.2848x; 1.1940x over previous
"""Trainium2 Bass kernel: grouped similarity-gating normalization.

Reference computation (per batch b, group g, cpg=64 channels, hw=784):
    means[c]  = mean_hw(x[c, :])
    s[hw]     = sum_c x[c, hw] * means[c]
    t         = (s - mean(s)) * rsqrt(var(s) + eps)
    gate      = sigmoid(t * weight[g] + bias[g])
    out[c,hw] = x[c, hw] * gate[hw]

Sharding: data-parallel over batch B=64 across 8 cores (8 batches/core).

Per-core layout: one SBUF tile [128, 4, 784] per batch holds channels
c = 4*p + j (p = partition, j = free chunk) -> contiguous 1.6MB DMAs, and
group(c) = c//64 = p//16, i.e. each group owns a 16-partition band.

v2 design (memory-regime; HBM floor ~54us/core with fp16 output):
  - input DMAs ride the scalar (ACT) HWDGE ring, output DMAs the sync
    (SP) ring -> no head-of-line blocking between loads and stores.
  - channel sums: j0 via one DVE reduce, j1-3 via ACT Copy+accum_out
    (copy lives in the sigmoid table set -> no ACT table swaps at all).
  - s via PE: 4 accumulating fp32r matmuls with lhsT = indicator*means.
  - mean/var of s via DVE bn_stats/bn_aggr straight from PSUM (exact:
    4 equal-count sub-groups of 196).
  - rstd on DVE: int32 magic-constant seed (shift/xor/add) + 3 Newton
    iterations -> no Sqrt on ACT, so zero ACT_TABLE_LOADs in steady
    state (all ACT funcs sit in sigmoid_and_others).
  - gate = sigmoid(s*a + c) one ACT op with per-partition scale/bias.
  - gating multiply split DVE (j0,j1) / GpSimd (j2,j3), each as ONE
    broadcast-middle-dim tensor_tensor writing fp16 -> output HBM
    traffic halved; host upcasts to fp32 (rel err ~4e-4 << 2e-2 gate).
"""

import sys

if "/opt/trn_rl_repo" not in sys.path:
    sys.path.insert(0, "/opt/trn_rl_repo")

from contextlib import ExitStack

import numpy as np

import concourse.bacc as bacc
import concourse.tile as tile
from concourse import mybir
from concourse.bass_utils import run_bass_kernel_spmd

B, C, H, W = 64, 512, 28, 28
G = 8
HW = H * W          # 784
NCORES = 8
BLOC = B // NCORES  # 8 batches per core
NP = 128            # SBUF partitions
NJ = C // NP        # 4 channel chunks per partition (c = NJ*p + j)
PBAND = NP // G     # 16 partitions per group
EPS = 1e-5
F32 = mybir.dt.float32
F32R = mybir.dt.float32r
F16 = mybir.dt.float16
I32 = mybir.dt.int32
MMCHUNK = 512       # max fp32 moving free dim per matmul (PSUM bank)
PREF = 4            # input prefetch depth (batches)
NR_ITERS = 2        # Newton iterations for rsqrt
MAGIC = 0x5F3759DF  # rsqrt seed constant
# lhsT drops the 1/HW: t = (s-mu)/sqrt(var+eps) is scale-invariant, so use
# raw channel sums as weights and scale eps by HW^2 to match exactly.
EPS_EFF = float(HW) * float(HW) * EPS

# implementation knobs (bisectable)
N_DVE_SUMS = 2      # channel-sum j's done by one DVE reduce (rest: ACT copies)
LHST_ENGINE = "dve"  # "dve" (tensor_scalar) or "act" (Copy w/ scale)
MUL_SPLIT = 4        # j's in the DVE gating mul (rest on GpSimd)

_cache: dict = {}


def _emit(tc, nc, xs, m8, wv, bv, ys):
    AF = mybir.ActivationFunctionType
    OP = mybir.AluOpType
    with ExitStack() as ctx:
        consts = ctx.enter_context(tc.tile_pool(name="consts", bufs=1))
        xpool = ctx.enter_context(tc.tile_pool(name="xpool", bufs=BLOC))
        spool = ctx.enter_context(tc.tile_pool(name="spool", bufs=4))
        lpool = ctx.enter_context(tc.tile_pool(name="lpool", bufs=3))
        cpool = ctx.enter_context(tc.tile_pool(name="cpool", bufs=2))
        gpool = ctx.enter_context(tc.tile_pool(name="gpool", bufs=3))
        vpool = ctx.enter_context(tc.tile_pool(name="vpool", bufs=20))
        spsum = ctx.enter_context(tc.tile_pool(name="spsum", bufs=2, space="PSUM"))
        opool = ctx.enter_context(tc.tile_pool(name="opool", bufs=3))

        # M16[p, q] = (p//PBAND == q//PBAND) 0/1 indicator (exact in fp32r);
        # wv/bv are the 16x-replicated per-partition weight/bias columns.
        m16r_sb = consts.tile([NP, NP], F32R)
        nc.sync.dma_start(out=m16r_sb[:], in_=m8[:].bitcast(F32R))
        wv_sb = consts.tile([NP, 1], F32)
        nc.sync.dma_start(out=wv_sb[:], in_=wv[:])
        bv_sb = consts.tile([NP, 1], F32)
        nc.sync.dma_start(out=bv_sb[:], in_=bv[:])
        # dummy sigmoid so ACT's one table load is the sigmoid set (which
        # also holds copy/square) before real work arrives
        warm = consts.tile([NP, 1], F32)
        nc.vector.memset(warm[:], 0.0)
        nc.scalar.activation(out=warm[:], in_=warm[:], func=AF.Sigmoid)

        xts = {}
        state = {}

        def dma_in(b):
            # two halves so phase1 can start on j0/j1 at half-arrival;
            # scalar (ACT) ring: never blocks on xpool with bufs=BLOC
            xt = xpool.tile([NP, NJ, HW], F32R)
            nc.scalar.dma_start(out=xt[:, 0:2, :], in_=xs[b, :, 0:2, :])
            nc.scalar.dma_start(out=xt[:, 2:4, :], in_=xs[b, :, 2:4, :])
            xts[b] = xt

        def phase1(b):
            # channel sums + masked lhsT (pre-matmul per-batch prep)
            xt = xts[b]
            xf = lambda ap: ap.bitcast(F32)
            sums = spool.tile([NP, NJ], F32, tag="sums")
            if N_DVE_SUMS:
                nc.vector.reduce_sum(
                    out=sums[:, 0:N_DVE_SUMS],
                    in_=xf(xt[:, 0:N_DVE_SUMS, :]),
                    axis=mybir.AxisListType.X,
                )
            cps = cpool.tile([NP, HW], F32, tag="cps")
            for j in range(N_DVE_SUMS, NJ):
                nc.scalar.activation(
                    out=cps[:], in_=xf(xt[:, j, :]), func=AF.Copy,
                    accum_out=sums[:, j : j + 1],
                )
            # lhsT[p, j, q] = indicator[p, q] * sums[p, j] (masked raw sums)
            lhsT = lpool.tile([NP, NJ, NP], F32R, tag="lhsT")
            for j in range(NJ):
                if LHST_ENGINE == "dve":
                    nc.vector.tensor_scalar_mul(
                        lhsT[:, j, :], m16r_sb[:], sums[:, j : j + 1]
                    )
                else:
                    nc.scalar.activation(
                        out=lhsT[:, j, :], in_=xf(m16r_sb[:]), func=AF.Copy,
                        scale=sums[:, j : j + 1],
                    )
            state[b] = lhsT

        def phase2(b):
            # s replicated onto each group's 16-partition band (M=128 free)
            xt = xts[b]
            lhsT = state[b]
            ps = spsum.tile([NP, HW], F32)
            for c0 in range(0, HW, MMCHUNK):
                c1 = min(c0 + MMCHUNK, HW)
                for j in range(NJ):
                    nc.tensor.matmul(
                        ps[:, c0:c1], lhsT[:, j, :], xt[:, j, c0:c1],
                        start=(j == 0), stop=(j == NJ - 1),
                    )
            state[b] = ps

        def phase3(b):
            # stats from PSUM + rstd on DVE + gate on ACT
            ps = state[b]
            st6 = vpool.tile([NP, 2, 6], F32, tag="st6")
            nc.vector.bn_stats(st6[:, 0, :], ps[:, 0 : HW // 2])
            nc.vector.bn_stats(st6[:, 1, :], ps[:, HW // 2 : HW])
            mv = vpool.tile([NP, 2], F32, tag="mv")
            nc.vector.bn_aggr(mv[:], st6[:])
            u = vpool.tile([NP, 1], F32, tag="u")
            nc.vector.tensor_scalar_add(u[:], mv[:, 1:2], EPS_EFF)
            # y0 = bits(MAGIC - (bits(u) >> 1)); K - t = (t ^ -1) + (K + 1)
            y = vpool.tile([NP, 1], F32, tag="y")
            yi = y[:].bitcast(I32)
            nc.vector.tensor_scalar(
                out=yi, in0=u[:].bitcast(I32), scalar1=1, scalar2=None,
                op0=OP.logical_shift_right,
            )
            nc.vector.tensor_scalar(
                out=yi, in0=yi, scalar1=-1, scalar2=None, op0=OP.bitwise_xor
            )
            nc.vector.tensor_scalar(
                out=yi, in0=yi, scalar1=MAGIC + 1, scalar2=None, op0=OP.add
            )
            h = vpool.tile([NP, 1], F32, tag="h")
            nc.vector.tensor_scalar_mul(h[:], u[:], -0.5)
            t = vpool.tile([NP, 1], F32, tag="t")
            for _ in range(NR_ITERS):
                nc.vector.tensor_mul(t[:], y[:], y[:])
                nc.vector.tensor_mul(t[:], t[:], h[:])
                nc.vector.scalar_tensor_tensor(
                    out=y[:], in0=t[:], scalar=1.5, in1=y[:],
                    op0=OP.add, op1=OP.mult,
                )
            a_t = vpool.tile([NP, 1], F32, tag="a_t")
            nc.vector.tensor_mul(a_t[:], y[:], wv_sb[:])
            nmu = vpool.tile([NP, 1], F32, tag="nmu")
            nc.vector.tensor_scalar_mul(nmu[:], mv[:, 0:1], -1.0)
            c_t = vpool.tile([NP, 1], F32, tag="c_t")
            nc.vector.scalar_tensor_tensor(
                out=c_t[:], in0=nmu[:], scalar=a_t[:], in1=bv_sb[:],
                op0=OP.mult, op1=OP.add,
            )
            gate = gpool.tile([NP, HW], F32, tag="gate")
            nc.scalar.activation(
                out=gate[:], in_=ps[:, 0:HW], func=AF.Sigmoid, bias=c_t[:],
                scale=a_t[:],
            )
            state[b] = gate

        def phase4(b):
            # gating multiply (fp16 out, 2x DVE fast path) + sync-ring store
            xt = xts.pop(b)
            gate = state.pop(b)
            xf = lambda ap: ap.bitcast(F32)
            ot = opool.tile([NP, NJ, HW], F16)
            gb = lambda n: gate[:].unsqueeze(1).to_broadcast([NP, n, HW])
            k = MUL_SPLIT
            nc.vector.tensor_mul(ot[:, 0:k, :], xf(xt[:, 0:k, :]), gb(k))
            if k < NJ:
                nc.gpsimd.tensor_mul(
                    ot[:, k:NJ, :], xf(xt[:, k:NJ, :]), gb(NJ - k)
                )
                nc.sync.dma_start(out=ys[b, :, 0:k, :], in_=ot[:, 0:k, :])
                nc.sync.dma_start(out=ys[b, :, k:NJ, :], in_=ot[:, k:NJ, :])
            else:
                nc.sync.dma_start(out=ys[b], in_=ot[:])
            if b + PREF < BLOC:
                dma_in(b + PREF)

        # software-pipelined emission: each engine's stream sees work in
        # data-readiness order, so in-order engines never head-of-line block
        for b in range(min(PREF, BLOC)):
            dma_in(b)
        phase1(0)
        phase2(0)
        for b in range(BLOC):
            if b + 1 < BLOC:
                phase1(b + 1)
            phase3(b)
            if b + 1 < BLOC:
                phase2(b + 1)
            phase4(b)


def _build_nc():
    nc = bacc.Bacc("TRN2", debug=False)
    xs = nc.dram_tensor("xs", [BLOC, NP, NJ, HW], F32R, kind="ExternalInput")
    m8 = nc.dram_tensor("m8", [NP, NP], F32, kind="ExternalInput")
    wv = nc.dram_tensor("wv", [NP, 1], F32, kind="ExternalInput")
    bv = nc.dram_tensor("bv", [NP, 1], F32, kind="ExternalInput")
    ys = nc.dram_tensor("ys", [BLOC, NP, NJ, HW], F16, kind="ExternalOutput")
    with tile.TileContext(nc) as tc:
        _emit(tc, nc, xs, m8, wv, bv, ys)
    nc.compile()
    return nc


def get_nc():
    if "nc" not in _cache:
        _cache["nc"] = _build_nc()
    return _cache["nc"]


def make_in_maps(x, weight, bias):
    x = np.ascontiguousarray(np.asarray(x, dtype=np.float32))
    weight = np.asarray(weight, dtype=np.float32).reshape(G)
    bias = np.asarray(bias, dtype=np.float32).reshape(G)
    # [core, b, p, j, hw] with c = NJ*p + j
    xs = x.reshape(NCORES, BLOC, NP, NJ, HW)
    band = np.arange(NP) // PBAND
    m8 = (band[:, None] == band[None, :]).astype(np.float32)  # [NP, NP] indicator
    wv = np.ascontiguousarray(np.repeat(weight, PBAND)[:, None])
    bv = np.ascontiguousarray(np.repeat(bias, PBAND)[:, None])
    return [
        {"xs": np.ascontiguousarray(xs[i]), "m8": m8, "wv": wv, "bv": bv}
        for i in range(NCORES)
    ]


def run(x, weight, bias, trace=False, **spmd_kwargs):
    nc = get_nc()
    in_maps = make_in_maps(x, weight, bias)
    res = run_bass_kernel_spmd(
        nc, in_maps, core_ids=list(range(NCORES)), trace=trace, **spmd_kwargs
    )
    out = np.stack(
        [res.results[i]["ys"].astype(np.float32) for i in range(NCORES)]
    )
    return out.reshape(B, C, H, W), res


def kernel(x, weight, bias, groups=G, **_ignored):
    assert int(groups) == G
    out, _ = run(x, weight, bias, trace=False)
    return out
